# revision 19
# baseline (speedup 1.0000x reference)
"""Trainium2 Bass kernel for nn_End2EndTongueROI_Dynamic_NMS.

Key algebraic facts used (verified against the reference):
  - Greedy NMS always keeps the top-scored box first and fi=argmax(keep)=0,
    so the whole top-k/NMS tail reduces to argmax(score) over 8400 anchors.
  - score's /max(maskness) normalization and /32 mean are positive scalings
    shared by all anchors -> argmax-invariant -> dropped on device.
  - Both resizes are linear: expressed as matmuls with exact f32 weight
    matrices replicated from jax.image.resize's compute_weight_mat.
  - The final rect is data-dependent but narrow; each core computes its
    270-row shard restricted to a dynamic 516-column window that covers the
    rect. Everything outside the window is exactly 0 in the reference output
    and the PJRT path pre-zeroes/donates output buffers (documented contract:
    "kernels that don't write every element rely on that"), so only the
    window is written. A host-side check falls back to exact numpy if the
    rect ever exceeds the window (impossible for in-distribution inputs).

Sharding: H0=2160 rows split 8 x 270. Score fusion + argmax tail is tiny and
fully replicated per core (no collectives needed).
"""
import numpy as np

import concourse.bacc as bacc
import concourse.bass as bass
import concourse.mybir as mybir
import concourse.tile as tile
from concourse import bass_isa, bass_utils

F32 = mybir.dt.float32
I32 = mybir.dt.int32
U32 = mybir.dt.uint32

N_CORES = 8
H0, W0 = 2160, 3840
IMGSZ = 640
MASK_THR = 0.72
NANCH, NC_COL = 8400, 37
ROWS = H0 // N_CORES          # 270 rows per core
SROWS = 82                    # s640 row window per core
MROWS = 24                    # m160 row window per core (padded)
WWIN = 516                    # output column window (6*86)
SWIN = 88                     # s column window feeding WWIN
NPP = 66                      # anchors per partition (66*128 = 8448 >= 8400)
# sentinel for the argmin-over-winners trick; power of two > NANCH so that
# af - BIG and +BIG round-trip exactly in f32 (af < 2^14, ulp stays <= 1)
BIG = 16384.0


# ---------------------------------------------------------------------------
# host-side resize weights (exact replica of jax.image.resize bilinear)
# ---------------------------------------------------------------------------

def _weight_mat(in_size, out_size):
    dt = np.float32
    scale = dt(out_size / in_size)
    inv_scale = dt(1.0) / scale
    sample_f = (np.arange(out_size, dtype=dt) + dt(0.5)) * inv_scale - dt(0.5)
    x = np.abs(sample_f[None, :] - np.arange(in_size, dtype=dt)[:, None])
    w = np.maximum(dt(0), dt(1) - x).astype(dt)
    tot = w.sum(axis=0, keepdims=True).astype(dt)
    w = np.where(np.abs(tot) > 1000.0 * np.finfo(np.float32).eps,
                 w / np.where(tot != 0, tot, 1), 0).astype(dt)
    ok = (sample_f >= -0.5) & (sample_f <= in_size - 0.5)
    return np.where(ok[None, :], w, 0).astype(dt)


def _host_consts():
    """Per-core constant tensors. Returns (shared, percore_list)."""
    Ah = _weight_mat(160, IMGSZ)      # [160, 640]
    Aw = _weight_mat(160, IMGSZ)      # [160, 640]
    Vh = _weight_mat(IMGSZ, H0)       # [640, 2160]
    Vw = _weight_mat(IMGSZ, W0)       # [640, 3840]

    vwpad = np.zeros((642, W0), np.float32)
    vwpad[1:641] = Vw

    ones1 = np.ones((1, 128), np.float32)
    id1 = np.ones((1, 1), np.float32)

    percore = []
    for c in range(N_CORES):
        r0 = ROWS * c
        vh_sl = Vh[:, r0:r0 + ROWS]
        nz = np.where(vh_sl.any(axis=1))[0]
        ra = min(int(nz.min()), IMGSZ - SROWS)
        r82 = np.ascontiguousarray(vh_sl[ra:ra + SROWS, :])     # [82, 270]

        ah_sl = Ah[:, ra:ra + SROWS]                            # [160, 82]
        nzh = np.where(ah_sl.any(axis=1))[0]
        ha = min(int(nzh.min()), 160 - MROWS)
        ahst = np.ascontiguousarray(ah_sl[ha:ha + MROWS, :])    # [24, 82]

        riota = np.zeros((128, 3), np.float32)
        for t in range(3):
            vals = r0 + 128 * t + np.arange(128)
            vals = np.where(vals < r0 + ROWS, vals, -1.0e9)
            riota[:, t] = vals

        percore.append(dict(ra=ra, ha=ha, r82=r82, ahst=ahst, riota=riota))

    shared = dict(awt=np.ascontiguousarray(Aw), vwpad=vwpad, ones1=ones1,
                  id1=id1)
    return shared, percore


# ---------------------------------------------------------------------------
# device program (identical for all cores; per-core data comes via inputs)
# ---------------------------------------------------------------------------

def _build_nc(stage=99, reps=1):
    nc = bacc.Bacc("TRN2", target_bir_lowering=False, debug=False,
                   enable_asserts=False, num_devices=N_CORES)

    pred_d = nc.dram_tensor("pred", [NANCH, NC_COL], F32, kind="ExternalInput")
    x_d = nc.dram_tensor("xs", [3, ROWS, W0], F32, kind="ExternalInput")
    protos_d = nc.dram_tensor("protos", [32, MROWS * 160], F32, kind="ExternalInput")
    ahst_d = nc.dram_tensor("ahst", [MROWS, SROWS], F32, kind="ExternalInput")
    awt_d = nc.dram_tensor("awt", [160, IMGSZ], F32, kind="ExternalInput")
    r82_d = nc.dram_tensor("r82", [SROWS, ROWS], F32, kind="ExternalInput")
    vwpad_d = nc.dram_tensor("vwpad", [642, W0], F32, kind="ExternalInput")
    ones1_d = nc.dram_tensor("ones1", [1, 128], F32, kind="ExternalInput")
    id1_d = nc.dram_tensor("id1", [1, 1], F32, kind="ExternalInput")
    riota_d = nc.dram_tensor("riota", [128, 3], F32, kind="ExternalInput")

    out_d = nc.dram_tensor("out", [3, ROWS, W0], F32, kind="ExternalOutput")
    meta_d = nc.dram_tensor("meta", [1, 8], F32, kind="ExternalOutput")

    with tile.TileContext(nc) as tc:
        for rep in range(reps):
            _program(nc, tc, pred_d, x_d, protos_d, ahst_d, awt_d, r82_d,
                     vwpad_d, ones1_d, id1_d, riota_d, out_d, meta_d, stage,
                     rep)
    nc.compile()
    return nc


def _program(nc, tc, pred_d, x_d, protos_d, ahst_d, awt_d, r82_d, vwpad_d,
             ones1_d, id1_d, riota_d, out_d, meta_d, stage=99, rep=0):
    AF = mybir.ActivationFunctionType
    OP = mybir.AluOpType
    AX = mybir.AxisListType
    import contextlib
    ctx = contextlib.ExitStack()

    sb = ctx.enter_context(tc.tile_pool(name="sb", bufs=1))
    ps = ctx.enter_context(tc.tile_pool(name="ps", bufs=1, space=bass.MemorySpace.PSUM))

    _bias_cache = {}

    def cbias(val):
        if val not in _bias_cache:
            t = sb.tile([128, 1], F32, tag=f"cb{len(_bias_cache)}")
            nc.vector.memset(t[:, :], val)
            _bias_cache[val] = t
        return _bias_cache[val]

    def act(out_ap, in_ap, func, bias=0.0, scale=1.0):
        nparts = in_ap.shape[0]
        nc.scalar.activation(out_ap, in_ap, func,
                             bias=cbias(float(bias))[0:nparts, :],
                             scale=scale)

    # ---------------- stage S: score fusion + argmax ----------------
    # anchor a = p*66 + n; partition 127 cols >= 18 are uninitialized pad
    # (8400 = 127*66 + 18). The pad garbage is neutralized later with
    # affine_select fills (a memset + overlapping DMA combination wedges the
    # HW even though CoreSim accepts it).
    P2 = sb.tile([128, NPP * NC_COL], F32, tag="P2")
    # zero exactly the pad slots via DMA (compute engines cannot target a
    # lone partition 127; vwpad row 0 is all zeros, reuse it as a source)
    nc.sync.dma_start(P2[127:128, 18 * NC_COL:],
                      vwpad_d.ap()[0:1, 0:(NPP - 18) * NC_COL])
    nc.sync.dma_start(
        P2[0:127, :],
        pred_d.ap()[0:127 * NPP, :].rearrange("(p n) c -> p (n c)", n=NPP))
    nc.sync.dma_start(
        P2[127:128, 0:18 * NC_COL],
        pred_d.ap()[127 * NPP:NANCH, :].rearrange("(p n) c -> p (n c)", p=1))

    P3 = P2[:, :].rearrange("p (n c) -> p n c", c=NC_COL)   # [128, 66, 37]

    sg = sb.tile([128, NPP], F32, tag="sg")
    act(sg[:, :], P3[:, :, 4], AF.Sigmoid)
    s1 = sb.tile([128, NPP], F32, tag="s1")
    act(s1[:, :], sg[:, :], AF.Relu, bias=-0.5)

    mk = sb.tile([128, NPP], F32, tag="mk")
    nc.vector.tensor_reduce(mk[:, :], P3[:, :, 5:NC_COL], AX.X, OP.add,
                            apply_absolute_value=True)

    gm1 = sb.tile([128, 1], F32, tag="gm1")
    nc.vector.tensor_reduce(gm1[:, :], P3[:, :, 0:4], AX.XY, OP.max)
    gmax = sb.tile([128, 1], F32, tag="gmax")
    nc.gpsimd.partition_all_reduce(gmax[:, :], gm1[:, :], 128,
                                   bass_isa.ReduceOp.max)
    fsc = sb.tile([128, 1], F32, tag="fsc")
    nc.vector.tensor_scalar(fsc[:, :], gmax[:, :], 1.2, 639.0, OP.is_le, OP.mult)
    nc.vector.tensor_scalar(fsc[:, :], fsc[:, :], 1.0, None, OP.add)

    cxf = sb.tile([128, NPP], F32, tag="cxf")
    cyf = sb.tile([128, NPP], F32, tag="cyf")
    nc.vector.tensor_scalar(cxf[:, :], P3[:, :, 0], fsc[:, :], None, OP.mult)
    nc.vector.tensor_scalar(cyf[:, :], P3[:, :, 1], fsc[:, :], None, OP.mult)
    dxa = sb.tile([128, NPP], F32, tag="dxa")
    dya = sb.tile([128, NPP], F32, tag="dya")
    act(dxa[:, :], cxf[:, :], AF.Abs, bias=-320.0)
    act(dya[:, :], cyf[:, :], AF.Abs, bias=-320.0)
    uxy = sb.tile([128, NPP], F32, tag="uxy")
    nc.vector.tensor_tensor(uxy[:, :], dxa[:, :], dya[:, :], OP.add)
    cw = sb.tile([128, NPP], F32, tag="cw")
    act(cw[:, :], uxy[:, :], AF.Relu, bias=1.0, scale=-1.0 / 640.0)
    cwf = sb.tile([128, NPP], F32, tag="cwf")
    nc.vector.tensor_scalar(cwf[:, :], cw[:, :], 0.5, 0.5, OP.mult, OP.add)

    score = sb.tile([128, NPP], F32, tag="score")
    nc.vector.tensor_scalar(score[:, :], s1[:, :], 0.001, None, OP.add)
    nc.vector.tensor_tensor(score[:, :], score[:, :], mk[:, :], OP.mult)
    nc.vector.tensor_tensor(score[:, :], score[:, :], cwf[:, :], OP.mult)

    vmax8 = sb.tile([128, 8], F32, tag="vmax8")
    vidx8 = sb.tile([128, 8], U32, tag="vidx8")
    nc.vector.max_with_indices(vmax8[:, :], vidx8[:, :], score[:, :])

    gsc = sb.tile([128, 1], F32, tag="gsc")
    nc.gpsimd.partition_all_reduce(gsc[:, :], vmax8[:, 0:1], 128,
                                   bass_isa.ReduceOp.max)

    pio = sb.tile([128, 1], I32, tag="pio")
    nc.gpsimd.iota(pio[:, :], [[0, 1]], channel_multiplier=NPP)
    piof = sb.tile([128, 1], F32, tag="piof")
    nc.vector.tensor_copy(piof[:, :], pio[:, :])
    vif = sb.tile([128, 1], F32, tag="vif")
    nc.vector.tensor_copy(vif[:, :], vidx8[:, 0:1])
    af = sb.tile([128, 1], F32, tag="af")
    nc.vector.tensor_tensor(af[:, :], piof[:, :], vif[:, :], OP.add)

    wm = sb.tile([128, 1], F32, tag="wm")
    nc.vector.tensor_scalar(wm[:, :], vmax8[:, 0:1], gsc[:, :], None, OP.is_ge)
    cand = sb.tile([128, 1], F32, tag="cand")
    nc.vector.tensor_scalar(cand[:, :], af[:, :], BIG, None, OP.subtract)
    nc.vector.tensor_tensor(cand[:, :], cand[:, :], wm[:, :], OP.mult)
    nc.vector.tensor_scalar(cand[:, :], cand[:, :], BIG, -1.0, OP.add, OP.mult)
    mxn = sb.tile([128, 1], F32, tag="mxn")
    nc.gpsimd.partition_all_reduce(mxn[:, :], cand[:, :], 128,
                                   bass_isa.ReduceOp.max)
    a_f = sb.tile([128, 1], F32, tag="a_f")
    nc.vector.tensor_scalar(a_f[:, :], mxn[:, :], -1.0, None, OP.mult)
    a_i = sb.tile([1, 1], I32, tag="a_i")
    nc.vector.tensor_copy(a_i[0:1, :], a_f[0:1, :])

    if stage <= 1:
        metas = sb.tile([1, 8], F32, tag="metas")
        nc.vector.memset(metas[:, :], 0.0)
        nc.vector.tensor_copy(metas[0:1, 0:1], a_f[0:1, :])
        nc.sync.dma_start(meta_d.ap(), metas[:, :])
        ctx.close()
        return

    # ---------------- stage G: gather winner row + box ----------------
    row1 = sb.tile([1, NC_COL], F32, tag="row1")
    with nc.gpsimd.register(f"aoff{rep}") as areg:
        nc.gpsimd.reg_load(areg, a_i[0:1, 0:1])
        aoff = nc.gpsimd.snap(areg, min_val=0, max_val=NANCH - 1)
        nc.gpsimd.dma_start(row1[:, :], pred_d.ap()[bass.ds(aoff, 1), :])

    ones1 = sb.tile([1, 128], F32, tag="ones1")
    nc.sync.dma_start(ones1[:, :], ones1_d.ap())
    id1 = sb.tile([1, 1], F32, tag="id1")
    nc.sync.dma_start(id1[:, :], id1_d.ap())

    psB = ps.tile([128, NC_COL], F32, tag="ps")
    nc.tensor.matmul(psB[:, :], ones1[:, :], row1[:, :], start=True, stop=True)
    bc37 = sb.tile([128, NC_COL], F32, tag="bc37")
    nc.scalar.copy(bc37[:, :], psB[:, :])

    psT = ps.tile([32, 1], F32, tag="ps")
    nc.tensor.transpose(psT[:, :], row1[:, 5:NC_COL], id1[:, :])
    coefT = sb.tile([32, 1], F32, tag="coefT")
    nc.scalar.copy(coefT[:, :], psT[:, :])

    # box -> fb (full-res rect bounds), [128,1] broadcast columns
    halfw = sb.tile([128, 1], F32, tag="halfw")
    halfh = sb.tile([128, 1], F32, tag="halfh")
    nc.vector.tensor_scalar(halfw[:, :], bc37[:, 2:3], 0.5, None, OP.mult)
    nc.vector.tensor_scalar(halfh[:, :], bc37[:, 3:4], 0.5, None, OP.mult)

    def clipped(dst, src_col, half, op, sxy):
        t = sb.tile([128, 1], F32, tag=dst + "_t")
        nc.vector.tensor_tensor(t[:, :], bc37[:, src_col:src_col + 1],
                                half[:, :], op)
        nc.vector.tensor_scalar(t[:, :], t[:, :], 0.0, float(IMGSZ - 1),
                                OP.max, OP.min)
        o = sb.tile([128, 1], F32, tag=dst)
        nc.vector.tensor_scalar(o[:, :], t[:, :], sxy, None, OP.mult)
        return o

    SX, SY = W0 / IMGSZ, H0 / IMGSZ
    fb0 = clipped("fb0", 0, halfw, OP.subtract, SX)
    fb1 = clipped("fb1", 1, halfh, OP.subtract, SY)
    fb2 = clipped("fb2", 0, halfw, OP.add, SX)
    fb3 = clipped("fb3", 1, halfh, OP.add, SY)

    # window base: m = clamp(round(fb0/6 - 1.5), 0, 554); c0 = 6m
    c0m = sb.tile([128, 1], F32, tag="c0m")
    nc.vector.tensor_scalar(c0m[:, :], fb0[:, :], 1.0 / 6.0, 1.5, OP.mult,
                            OP.subtract)
    nc.vector.tensor_scalar(c0m[:, :], c0m[:, :], 0.0, 554.0, OP.max, OP.min)
    m_i = sb.tile([1, 1], I32, tag="m_i")
    nc.vector.tensor_copy(m_i[0:1, :], c0m[0:1, :])
    c0_i = sb.tile([1, 1], I32, tag="c0_i")
    nc.vector.tensor_scalar(c0_i[:, :], m_i[:, :], 6, None, OP.mult)
    c0f1 = sb.tile([1, 1], F32, tag="c0f1")
    nc.vector.tensor_copy(c0f1[0:1, :], c0_i[0:1, :])
    psC = ps.tile([128, 1], F32, tag="ps")
    nc.tensor.matmul(psC[:, :], ones1[:, :], c0f1[:, :], start=True, stop=True)
    c0col = sb.tile([128, 1], F32, tag="c0col")
    nc.scalar.copy(c0col[:, :], psC[:, :])

    # meta output for host fallback check: [a, fb0..3, c0]
    metas = sb.tile([1, 8], F32, tag="metas")
    nc.vector.tensor_copy(metas[0:1, 0:1], a_f[0:1, :])
    nc.vector.tensor_copy(metas[0:1, 1:2], fb0[0:1, :])
    nc.vector.tensor_copy(metas[0:1, 2:3], fb1[0:1, :])
    nc.vector.tensor_copy(metas[0:1, 3:4], fb2[0:1, :])
    nc.vector.tensor_copy(metas[0:1, 4:5], fb3[0:1, :])
    nc.vector.tensor_copy(metas[0:1, 5:6], c0f1[0:1, :])
    nc.vector.tensor_copy(metas[0:1, 6:7], gsc[0:1, :])
    nc.vector.tensor_copy(metas[0:1, 7:8], fsc[0:1, :])
    nc.sync.dma_start(meta_d.ap(), metas[:, :])

    if stage <= 2:
        ctx.close()
        return

    # ---------------- stage M: mask pipeline ----------------
    protos = sb.tile([32, MROWS * 160], F32, tag="protos")
    nc.sync.dma_start(protos[:, :], protos_d.ap())
    ahst = sb.tile([MROWS, SROWS], F32, tag="ahst")
    nc.sync.dma_start(ahst[:, :], ahst_d.ap())
    awt_a = sb.tile([128, IMGSZ], F32, tag="awt_a")
    awt_b = sb.tile([32, IMGSZ], F32, tag="awt_b")
    nc.sync.dma_start(awt_a[:, :], awt_d.ap()[0:128, :])
    nc.sync.dma_start(awt_b[:, :], awt_d.ap()[128:160, :])
    r82 = sb.tile([SROWS, ROWS], F32, tag="r82")
    nc.sync.dma_start(r82[:, :], r82_d.ap())
    riota = sb.tile([128, 3], F32, tag="riota")
    nc.sync.dma_start(riota[:, :], riota_d.ap())

    m160f = sb.tile([1, MROWS * 160], F32, tag="m160f")
    HALF = MROWS * 160 // 2                            # 1920 = 4 banks
    for h in range(2):
        psM = ps.tile([1, HALF], F32, tag="psM", name=f"psM{rep}_{h}")
        for i in range(4):
            n0, n1 = 512 * i, min(512 * (i + 1), HALF)
            nc.tensor.matmul(psM[0:1, n0:n1], coefT,
                             protos[:, HALF * h + n0:HALF * h + n1],
                             start=True, stop=True)
        nc.scalar.copy(m160f[0:1, HALF * h:HALF * (h + 1)], psM[:, :])
    m160r = sb.tile([MROWS, 160], F32, tag="m160r")
    nc.sync.dma_start(
        m160r[:, :],
        m160f[:, :].rearrange("q (h w) -> (q h) w", w=160))

    # step1: P1T[w160, h82] = sum_h m160r[h, w] * ahst[h, j]
    psP1 = ps.tile([128, SROWS], F32, tag="ps")
    psP2 = ps.tile([32, SROWS], F32, tag="ps")
    nc.tensor.matmul(psP1[:, :], m160r[:, 0:128], ahst[:, :], start=True, stop=True)
    nc.tensor.matmul(psP2[:, :], m160r[:, 128:160], ahst[:, :], start=True, stop=True)
    p1 = sb.tile([128, SROWS], F32, tag="p1")
    p2 = sb.tile([32, SROWS], F32, tag="p2")
    nc.scalar.copy(p1[:, :], psP1[:, :])
    nc.scalar.copy(p2[:, :], psP2[:, :])

    # step2: m640rows[h82, j640] = sum_w P1T[w, h] * AwT[w, j]
    psQ = ps.tile([SROWS, IMGSZ], F32, tag="ps")
    for n0, n1 in ((0, 512), (512, 640)):
        nc.tensor.matmul(psQ[:, n0:n1], p1[:, :], awt_a[:, n0:n1],
                         start=True, stop=False)
        nc.tensor.matmul(psQ[:, n0:n1], p2[:, :], awt_b[:, n0:n1],
                         start=False, stop=True)

    s_pad = sb.tile([SROWS, 642], F32, tag="s_pad")
    act(s_pad[:, 1:641], psQ[:, :], AF.Sigmoid)
    nc.vector.tensor_copy(s_pad[:, 0:1], s_pad[:, 1:2])
    nc.vector.tensor_copy(s_pad[:, 641:642], s_pad[:, 640:641])

    if stage <= 3:
        ctx.close()
        return

    # dynamic windows
    s_win = sb.tile([SROWS, SWIN], F32, tag="s_win")
    vww = sb.tile([SWIN, WWIN], F32, tag="vww")
    xw = [sb.tile([128, 3 * WWIN], F32, tag=f"xw{t}", name=f"xw{rep}_{t}")
          for t in range(3)]
    RT = [128, 128, ROWS - 256]
    with nc.gpsimd.register(f"mo{rep}") as mreg, \
            nc.gpsimd.register(f"co_{rep}") as creg:
        nc.gpsimd.reg_load(mreg, m_i[0:1, 0:1])
        nc.gpsimd.reg_load(creg, c0_i[0:1, 0:1])
        mo = nc.gpsimd.snap(mreg, min_val=0, max_val=554)
        co = nc.gpsimd.snap(creg, min_val=0, max_val=W0 - WWIN)
        nc.gpsimd.dma_start(s_win[:, :], s_pad[:, bass.ds(mo, SWIN)])
        nc.gpsimd.dma_start(vww[:, :],
                            vwpad_d.ap()[bass.ds(mo, SWIN), bass.ds(co, WWIN)])
        xt = x_d.ap().transpose([1, 0, 2])      # [270, 3, 3840]
        for t in range(3):
            nc.gpsimd.dma_start(
                xw[t][0:RT[t], :].rearrange("p (c w) -> p c w", c=3),
                xt[128 * t:128 * t + RT[t], :, bass.ds(co, WWIN)])

    if stage <= 4:
        ctx.close()
        return

    # uT[i88, m270] = sum_p s_win[p, i] * r82[p, m]
    psU = ps.tile([SWIN, ROWS], F32, tag="ps")
    nc.tensor.matmul(psU[:, :], s_win[:, :], r82[:, :], start=True, stop=True)
    uT = sb.tile([SWIN, ROWS], F32, tag="uT")
    nc.scalar.copy(uT[:, :], psU[:, :])

    if stage <= 5:
        ctx.close()
        return

    # ---------------- stage O: threshold + rect + multiply ----------------
    xio = sb.tile([128, WWIN], I32, tag="xio")
    nc.gpsimd.iota(xio[:, :], [[1, WWIN]], channel_multiplier=0)
    xiof = sb.tile([128, WWIN], F32, tag="xiof")
    nc.vector.tensor_copy(xiof[:, :], xio[:, :])
    xcol = sb.tile([128, WWIN], F32, tag="xcol")
    nc.vector.tensor_scalar(xcol[:, :], xiof[:, :], c0col[:, :], None, OP.add)
    cma = sb.tile([128, WWIN], F32, tag="cma")
    nc.vector.tensor_scalar(cma[:, :], xcol[:, :], fb0[:, :], 255.0,
                            OP.is_ge, OP.mult)
    cmb = sb.tile([128, WWIN], F32, tag="cmb")
    nc.vector.tensor_scalar(cmb[:, :], xcol[:, :], fb2[:, :], None, OP.is_lt)
    cm255 = sb.tile([128, WWIN], F32, tag="cm255")
    nc.vector.tensor_tensor(cm255[:, :], cma[:, :], cmb[:, :], OP.mult)

    out_t = out_d.ap().transpose([1, 0, 2])         # [270, 3, 3840]
    for t in range(3):
        rt = RT[t]
        psW = ps.tile([128, WWIN], F32, tag="ps", name=f"psW{rep}_{t}")
        nc.tensor.matmul(psW[0:rt, 0:512], uT[:, 128 * t:128 * t + rt],
                         vww[:, 0:512], start=True, stop=True)
        nc.tensor.matmul(psW[0:rt, 512:WWIN], uT[:, 128 * t:128 * t + rt],
                         vww[:, 512:WWIN], start=True, stop=True)
        sgn = sb.tile([128, WWIN], F32, tag=f"sgn{t}")
        act(sgn[0:rt, :], psW[0:rt, :], AF.Sign, bias=-MASK_THR)
        rma = sb.tile([128, 1], F32, tag=f"rma{t}")
        nc.vector.tensor_scalar(rma[:, :], riota[:, t:t + 1], fb1[:, :], None,
                                OP.is_ge)
        rmb = sb.tile([128, 1], F32, tag=f"rmb{t}")
        nc.vector.tensor_scalar(rmb[:, :], riota[:, t:t + 1], fb3[:, :], None,
                                OP.is_lt)
        rm = sb.tile([128, 1], F32, tag=f"rm{t}")
        nc.vector.tensor_tensor(rm[:, :], rma[:, :], rmb[:, :], OP.mult)
        bm = sb.tile([128, WWIN], F32, tag=f"bm{t}")
        nc.vector.tensor_scalar(bm[0:rt, :], sgn[0:rt, :], 0.0, rm[0:rt, :],
                                OP.max, OP.mult)
        bm3 = sb.tile([128, WWIN], F32, tag=f"bm3{t}")
        nc.vector.tensor_tensor(bm3[0:rt, :], bm[0:rt, :], cm255[0:rt, :],
                                OP.mult)
        res = sb.tile([128, 3 * WWIN], F32, tag=f"res{t}")
        for ch in range(3):
            nc.vector.tensor_tensor(
                res[0:rt, WWIN * ch:WWIN * (ch + 1)],
                xw[t][0:rt, WWIN * ch:WWIN * (ch + 1)], bm3[0:rt, :], OP.mult)
        with nc.gpsimd.register(f"co{rep}_{t}") as creg:
            nc.gpsimd.reg_load(creg, c0_i[0:1, 0:1])
            co = nc.gpsimd.snap(creg, min_val=0, max_val=W0 - WWIN)
            nc.gpsimd.dma_start(
                out_t[128 * t:128 * t + rt, :, bass.ds(co, WWIN)],
                res[0:rt, :].rearrange("p (c w) -> p c w", c=3))

    ctx.close()


# ---------------------------------------------------------------------------
# host orchestration
# ---------------------------------------------------------------------------

_NC_CACHE = None


def _get_nc():
    global _NC_CACHE
    if _NC_CACHE is None:
        _NC_CACHE = _build_nc()
    return _NC_CACHE


def _numpy_fallback(x_raw, pred, proto):
    """Exact slow-path reference (only used if the rect exceeds the device
    window, which cannot happen for in-distribution inputs)."""
    p = pred[0]
    boxes, cls, coef = p[:, :4], p[:, 4], p[:, 5:]
    s1 = np.maximum(1.0 / (1.0 + np.exp(-cls)) - 0.5, 0) + np.float32(0.001)
    mk = np.abs(coef).sum(-1)
    f = np.float32(640.0 if boxes.max() <= 1.2 else 1.0)
    dxdy = np.abs(boxes[:, :2] * f - 320.0) / 320.0
    cw = np.maximum(1.0 - 0.5 * (dxdy[:, 0] + dxdy[:, 1]), 0.0)
    a = int(np.argmax(s1 * mk * (0.5 + 0.5 * cw)))
    fcoef = coef[a]
    cx, cy, w, h = boxes[a]
    xyxy = np.clip(np.array([cx - w / 2, cy - h / 2, cx + w / 2, cy + h / 2],
                            np.float32), 0.0, IMGSZ - 1)
    fb = xyxy * np.array([W0 / IMGSZ, H0 / IMGSZ, W0 / IMGSZ, H0 / IMGSZ],
                         np.float32)
    Ah = _weight_mat(160, IMGSZ)
    Aw = _weight_mat(160, IMGSZ)
    Vh = _weight_mat(IMGSZ, H0)
    Vw = _weight_mat(IMGSZ, W0)
    m160 = (fcoef @ proto[0].reshape(32, -1)).reshape(160, 160)
    m640 = Ah.T @ m160 @ Aw
    s640 = 1.0 / (1.0 + np.exp(-m640))
    m_orig = (Vh.T @ s640 @ Vw).astype(np.float32)
    ys = np.arange(H0, dtype=np.float32)[:, None]
    xs = np.arange(W0, dtype=np.float32)[None, :]
    rect = (xs >= fb[0]) & (xs < fb[2]) & (ys >= fb[1]) & (ys < fb[3])
    bm = ((m_orig > MASK_THR) & rect).astype(np.float32)
    return (np.clip(x_raw * 255.0, 0.0, 255.0) * bm[None, None]).astype(np.float32)


def kernel(x_raw, pred, proto):
    x_raw = np.ascontiguousarray(np.asarray(x_raw, dtype=np.float32))
    pred = np.ascontiguousarray(np.asarray(pred, dtype=np.float32))
    proto = np.ascontiguousarray(np.asarray(proto, dtype=np.float32))

    nc = _get_nc()
    shared, percore = _host_consts()
    pred2 = np.ascontiguousarray(pred[0])           # [8400, 37]
    proto2 = proto[0]                               # [32, 160, 160]

    in_maps = []
    for c in range(N_CORES):
        pc = percore[c]
        ha = pc["ha"]
        protos_sl = np.ascontiguousarray(
            proto2[:, ha:ha + MROWS, :].reshape(32, MROWS * 160))
        in_maps.append({
            "pred": pred2,
            "xs": np.ascontiguousarray(x_raw[0, :, ROWS * c:ROWS * (c + 1), :]),
            "protos": protos_sl,
            "ahst": pc["ahst"],
            "awt": shared["awt"],
            "r82": pc["r82"],
            "vwpad": shared["vwpad"],
            "ones1": shared["ones1"],
            "id1": shared["id1"],
            "riota": pc["riota"],
        })

    res = bass_utils.run_bass_kernel_spmd(nc, in_maps,
                                          core_ids=list(range(N_CORES)))

    meta = res.results[0]["meta"][0]
    fb0, fb1, fb2, fb3, c0 = meta[1], meta[2], meta[3], meta[4], meta[5]
    # fallback if the rect column span is not covered by the device window
    covered = (fb2 <= fb0) or (
        (c0 <= np.floor(fb0)) and (np.ceil(fb2) <= c0 + WWIN))
    if not covered:
        return _numpy_fallback(x_raw, pred, proto)

    out = np.concatenate([res.results[c]["out"] for c in range(N_CORES)],
                         axis=1)          # [3, 2160, 3840]
    return out[None]


if __name__ == "__main__":
    import jax
    with jax.default_device(jax.devices("cpu")[0]):
        import reference as R
        inputs = R.setup_inputs()
        inputs = {k: np.asarray(v) for k, v in inputs.items()}
    out = kernel(**inputs)
    ref = np.load("/tmp/ref_out.npy")
    print("absmax:", np.abs(out - ref).max())


# revision 20
# speedup vs baseline: 185.2873x; 185.2873x over previous
"""Trainium2 Bass kernel for nn_End2EndTongueROI_Dynamic_NMS.

Key algebraic facts used (verified against the reference):
  - Greedy NMS always keeps the top-scored box first and fi=argmax(keep)=0,
    so the whole top-k/NMS tail reduces to argmax(score) over 8400 anchors.
  - score's /max(maskness) normalization and /32 mean are positive scalings
    shared by all anchors -> argmax-invariant -> dropped on device.
  - Both resizes are linear: expressed as matmuls with exact f32 weight
    matrices replicated from jax.image.resize's compute_weight_mat.
  - The final rect is data-dependent but narrow; each core computes its
    270-row shard restricted to a dynamic 516-column window that covers the
    rect. Everything outside the window is exactly 0 in the reference output
    and the PJRT path pre-zeroes/donates output buffers (documented contract:
    "kernels that don't write every element rely on that"), so only the
    window is written. A host-side check falls back to exact numpy if the
    rect ever exceeds the window (impossible for in-distribution inputs).

Sharding: H0=2160 rows split 8 x 270. Score fusion + argmax tail is tiny and
fully replicated per core (no collectives needed).
"""
import numpy as np

import concourse.bacc as bacc
import concourse.bass as bass
import concourse.mybir as mybir
import concourse.tile as tile
from concourse import bass_isa, bass_utils

F32 = mybir.dt.float32
I32 = mybir.dt.int32
U32 = mybir.dt.uint32

N_CORES = 8
H0, W0 = 2160, 3840
IMGSZ = 640
MASK_THR = 0.72
NANCH, NC_COL = 8400, 37
ROWS = H0 // N_CORES          # 270 rows per core
SROWS = 82                    # s640 row window per core
MROWS = 24                    # m160 row window per core (padded)
WWIN = 516                    # output column window (6*86)
SWIN = 88                     # s column window feeding WWIN
NPP = 66                      # anchors per partition (66*128 = 8448 >= 8400)
# sentinel for the argmin-over-winners trick; power of two > NANCH so that
# af - BIG and +BIG round-trip exactly in f32 (af < 2^14, ulp stays <= 1)
BIG = 16384.0


# ---------------------------------------------------------------------------
# host-side resize weights (exact replica of jax.image.resize bilinear)
# ---------------------------------------------------------------------------

def _weight_mat(in_size, out_size):
    dt = np.float32
    scale = dt(out_size / in_size)
    inv_scale = dt(1.0) / scale
    sample_f = (np.arange(out_size, dtype=dt) + dt(0.5)) * inv_scale - dt(0.5)
    x = np.abs(sample_f[None, :] - np.arange(in_size, dtype=dt)[:, None])
    w = np.maximum(dt(0), dt(1) - x).astype(dt)
    tot = w.sum(axis=0, keepdims=True).astype(dt)
    w = np.where(np.abs(tot) > 1000.0 * np.finfo(np.float32).eps,
                 w / np.where(tot != 0, tot, 1), 0).astype(dt)
    ok = (sample_f >= -0.5) & (sample_f <= in_size - 0.5)
    return np.where(ok[None, :], w, 0).astype(dt)


def _host_consts():
    """Per-core constant tensors. Returns (shared, percore_list)."""
    Ah = _weight_mat(160, IMGSZ)      # [160, 640]
    Aw = _weight_mat(160, IMGSZ)      # [160, 640]
    Vh = _weight_mat(IMGSZ, H0)       # [640, 2160]
    Vw = _weight_mat(IMGSZ, W0)       # [640, 3840]

    vwpad = np.zeros((642, W0), np.float32)
    vwpad[1:641] = Vw

    ones1 = np.ones((1, 128), np.float32)
    id1 = np.ones((1, 1), np.float32)

    percore = []
    for c in range(N_CORES):
        r0 = ROWS * c
        vh_sl = Vh[:, r0:r0 + ROWS]
        nz = np.where(vh_sl.any(axis=1))[0]
        ra = min(int(nz.min()), IMGSZ - SROWS)
        r82 = np.ascontiguousarray(vh_sl[ra:ra + SROWS, :])     # [82, 270]

        ah_sl = Ah[:, ra:ra + SROWS]                            # [160, 82]
        nzh = np.where(ah_sl.any(axis=1))[0]
        ha = min(int(nzh.min()), 160 - MROWS)
        ahst = np.ascontiguousarray(ah_sl[ha:ha + MROWS, :])    # [24, 82]

        riota = np.zeros((128, 3), np.float32)
        for t in range(3):
            vals = r0 + 128 * t + np.arange(128)
            vals = np.where(vals < r0 + ROWS, vals, -1.0e9)
            riota[:, t] = vals

        percore.append(dict(ra=ra, ha=ha, r82=r82, ahst=ahst, riota=riota))

    shared = dict(awt=np.ascontiguousarray(Aw), vwpad=vwpad, ones1=ones1,
                  id1=id1)
    return shared, percore


# ---------------------------------------------------------------------------
# device program (identical for all cores; per-core data comes via inputs)
# ---------------------------------------------------------------------------

def _build_nc(stage=99, reps=1, loop_n=0):
    nc = bacc.Bacc("TRN2", target_bir_lowering=False, debug=False,
                   enable_asserts=False, num_devices=N_CORES)

    pred_d = nc.dram_tensor("pred", [NANCH, NC_COL], F32, kind="ExternalInput")
    x_d = nc.dram_tensor("xs", [3, ROWS, W0], F32, kind="ExternalInput")
    protos_d = nc.dram_tensor("protos", [32, MROWS * 160], F32, kind="ExternalInput")
    ahst_d = nc.dram_tensor("ahst", [MROWS, SROWS], F32, kind="ExternalInput")
    awt_d = nc.dram_tensor("awt", [160, IMGSZ], F32, kind="ExternalInput")
    r82_d = nc.dram_tensor("r82", [SROWS, ROWS], F32, kind="ExternalInput")
    vwpad_d = nc.dram_tensor("vwpad", [642, W0], F32, kind="ExternalInput")
    ones1_d = nc.dram_tensor("ones1", [1, 128], F32, kind="ExternalInput")
    id1_d = nc.dram_tensor("id1", [1, 1], F32, kind="ExternalInput")
    riota_d = nc.dram_tensor("riota", [128, 3], F32, kind="ExternalInput")

    out_d = nc.dram_tensor("out", [3, ROWS, W0], F32, kind="ExternalOutput")
    meta_d = nc.dram_tensor("meta", [1, 8], F32, kind="ExternalOutput")

    with tile.TileContext(nc) as tc:
        if loop_n:
            with tc.For_i(0, loop_n, 1):
                _program(nc, tc, pred_d, x_d, protos_d, ahst_d, awt_d, r82_d,
                         vwpad_d, ones1_d, id1_d, riota_d, out_d, meta_d,
                         stage, 0)
        else:
            for rep in range(reps):
                _program(nc, tc, pred_d, x_d, protos_d, ahst_d, awt_d, r82_d,
                         vwpad_d, ones1_d, id1_d, riota_d, out_d, meta_d,
                         stage, rep)
    nc.compile()
    return nc


def _program(nc, tc, pred_d, x_d, protos_d, ahst_d, awt_d, r82_d, vwpad_d,
             ones1_d, id1_d, riota_d, out_d, meta_d, stage=99, rep=0):
    AF = mybir.ActivationFunctionType
    OP = mybir.AluOpType
    AX = mybir.AxisListType
    import contextlib
    ctx = contextlib.ExitStack()

    sb = ctx.enter_context(tc.tile_pool(name="sb", bufs=1))
    ps = ctx.enter_context(tc.tile_pool(name="ps", bufs=1, space=bass.MemorySpace.PSUM))

    _bias_cache = {}

    def cbias(val):
        if val not in _bias_cache:
            t = sb.tile([128, 1], F32, tag=f"cb{len(_bias_cache)}")
            nc.vector.memset(t[:, :], val)
            _bias_cache[val] = t
        return _bias_cache[val]

    def act(out_ap, in_ap, func, bias=0.0, scale=1.0):
        nparts = in_ap.shape[0]
        nc.scalar.activation(out_ap, in_ap, func,
                             bias=cbias(float(bias))[0:nparts, :],
                             scale=scale)

    # ---------------- stage S: score fusion + argmax ----------------
    # anchor a = p*66 + n; partition 127 cols >= 18 are uninitialized pad
    # (8400 = 127*66 + 18). The pad garbage is neutralized later with
    # affine_select fills (a memset + overlapping DMA combination wedges the
    # HW even though CoreSim accepts it).
    P2 = sb.tile([128, NPP * NC_COL], F32, tag="P2")
    # zero exactly the pad slots via DMA (compute engines cannot target a
    # lone partition 127; vwpad row 0 is all zeros, reuse it as a source)
    nc.sync.dma_start(P2[127:128, 18 * NC_COL:],
                      vwpad_d.ap()[0:1, 0:(NPP - 18) * NC_COL])
    nc.sync.dma_start(
        P2[0:127, :],
        pred_d.ap()[0:127 * NPP, :].rearrange("(p n) c -> p (n c)", n=NPP))
    nc.sync.dma_start(
        P2[127:128, 0:18 * NC_COL],
        pred_d.ap()[127 * NPP:NANCH, :].rearrange("(p n) c -> p (n c)", p=1))

    P3 = P2[:, :].rearrange("p (n c) -> p n c", c=NC_COL)   # [128, 66, 37]

    sg = sb.tile([128, NPP], F32, tag="sg")
    act(sg[:, :], P3[:, :, 4], AF.Sigmoid)
    s1 = sb.tile([128, NPP], F32, tag="s1")
    act(s1[:, :], sg[:, :], AF.Relu, bias=-0.5)

    mk = sb.tile([128, NPP], F32, tag="mk")
    nc.vector.tensor_reduce(mk[:, :], P3[:, :, 5:NC_COL], AX.X, OP.add,
                            apply_absolute_value=True)

    gm1 = sb.tile([128, 1], F32, tag="gm1")
    nc.vector.tensor_reduce(gm1[:, :], P3[:, :, 0:4], AX.XY, OP.max)
    gmax = sb.tile([128, 1], F32, tag="gmax")
    nc.gpsimd.partition_all_reduce(gmax[:, :], gm1[:, :], 128,
                                   bass_isa.ReduceOp.max)
    fsc = sb.tile([128, 1], F32, tag="fsc")
    nc.vector.tensor_scalar(fsc[:, :], gmax[:, :], 1.2, 639.0, OP.is_le, OP.mult)
    nc.vector.tensor_scalar(fsc[:, :], fsc[:, :], 1.0, None, OP.add)

    cxf = sb.tile([128, NPP], F32, tag="cxf")
    cyf = sb.tile([128, NPP], F32, tag="cyf")
    nc.vector.tensor_scalar(cxf[:, :], P3[:, :, 0], fsc[:, :], None, OP.mult)
    nc.vector.tensor_scalar(cyf[:, :], P3[:, :, 1], fsc[:, :], None, OP.mult)
    dxa = sb.tile([128, NPP], F32, tag="dxa")
    dya = sb.tile([128, NPP], F32, tag="dya")
    act(dxa[:, :], cxf[:, :], AF.Abs, bias=-320.0)
    act(dya[:, :], cyf[:, :], AF.Abs, bias=-320.0)
    uxy = sb.tile([128, NPP], F32, tag="uxy")
    nc.vector.tensor_tensor(uxy[:, :], dxa[:, :], dya[:, :], OP.add)
    cw = sb.tile([128, NPP], F32, tag="cw")
    act(cw[:, :], uxy[:, :], AF.Relu, bias=1.0, scale=-1.0 / 640.0)
    cwf = sb.tile([128, NPP], F32, tag="cwf")
    nc.vector.tensor_scalar(cwf[:, :], cw[:, :], 0.5, 0.5, OP.mult, OP.add)

    score = sb.tile([128, NPP], F32, tag="score")
    nc.vector.tensor_scalar(score[:, :], s1[:, :], 0.001, None, OP.add)
    nc.vector.tensor_tensor(score[:, :], score[:, :], mk[:, :], OP.mult)
    nc.vector.tensor_tensor(score[:, :], score[:, :], cwf[:, :], OP.mult)

    vmax8 = sb.tile([128, 8], F32, tag="vmax8")
    vidx8 = sb.tile([128, 8], U32, tag="vidx8")
    nc.vector.max_with_indices(vmax8[:, :], vidx8[:, :], score[:, :])

    gsc = sb.tile([128, 1], F32, tag="gsc")
    nc.gpsimd.partition_all_reduce(gsc[:, :], vmax8[:, 0:1], 128,
                                   bass_isa.ReduceOp.max)

    pio = sb.tile([128, 1], I32, tag="pio")
    nc.gpsimd.iota(pio[:, :], [[0, 1]], channel_multiplier=NPP)
    piof = sb.tile([128, 1], F32, tag="piof")
    nc.vector.tensor_copy(piof[:, :], pio[:, :])
    vif = sb.tile([128, 1], F32, tag="vif")
    nc.vector.tensor_copy(vif[:, :], vidx8[:, 0:1])
    af = sb.tile([128, 1], F32, tag="af")
    nc.vector.tensor_tensor(af[:, :], piof[:, :], vif[:, :], OP.add)

    wm = sb.tile([128, 1], F32, tag="wm")
    nc.vector.tensor_scalar(wm[:, :], vmax8[:, 0:1], gsc[:, :], None, OP.is_ge)
    cand = sb.tile([128, 1], F32, tag="cand")
    nc.vector.tensor_scalar(cand[:, :], af[:, :], BIG, None, OP.subtract)
    nc.vector.tensor_tensor(cand[:, :], cand[:, :], wm[:, :], OP.mult)
    nc.vector.tensor_scalar(cand[:, :], cand[:, :], BIG, -1.0, OP.add, OP.mult)
    mxn = sb.tile([128, 1], F32, tag="mxn")
    nc.gpsimd.partition_all_reduce(mxn[:, :], cand[:, :], 128,
                                   bass_isa.ReduceOp.max)
    a_f = sb.tile([128, 1], F32, tag="a_f")
    nc.vector.tensor_scalar(a_f[:, :], mxn[:, :], -1.0, None, OP.mult)
    a_i = sb.tile([1, 1], I32, tag="a_i")
    nc.vector.tensor_copy(a_i[0:1, :], a_f[0:1, :])

    if stage <= 1:
        metas = sb.tile([1, 8], F32, tag="metas")
        nc.vector.memset(metas[:, :], 0.0)
        nc.vector.tensor_copy(metas[0:1, 0:1], a_f[0:1, :])
        nc.sync.dma_start(meta_d.ap(), metas[:, :])
        ctx.close()
        return

    # ---------------- stage G: gather winner row + box ----------------
    row1 = sb.tile([1, NC_COL], F32, tag="row1")
    with nc.gpsimd.register(f"aoff{rep}") as areg:
        nc.gpsimd.reg_load(areg, a_i[0:1, 0:1])
        aoff = nc.gpsimd.snap(areg, min_val=0, max_val=NANCH - 1)
        nc.gpsimd.dma_start(row1[:, :], pred_d.ap()[bass.ds(aoff, 1), :])

    ones1 = sb.tile([1, 128], F32, tag="ones1")
    nc.sync.dma_start(ones1[:, :], ones1_d.ap())
    id1 = sb.tile([1, 1], F32, tag="id1")
    nc.sync.dma_start(id1[:, :], id1_d.ap())

    psB = ps.tile([128, NC_COL], F32, tag="ps")
    nc.tensor.matmul(psB[:, :], ones1[:, :], row1[:, :], start=True, stop=True)
    bc37 = sb.tile([128, NC_COL], F32, tag="bc37")
    nc.scalar.copy(bc37[:, :], psB[:, :])

    psT = ps.tile([32, 1], F32, tag="ps")
    nc.tensor.transpose(psT[:, :], row1[:, 5:NC_COL], id1[:, :])
    coefT = sb.tile([32, 1], F32, tag="coefT")
    nc.scalar.copy(coefT[:, :], psT[:, :])

    # box -> fb (full-res rect bounds), [128,1] broadcast columns
    halfw = sb.tile([128, 1], F32, tag="halfw")
    halfh = sb.tile([128, 1], F32, tag="halfh")
    nc.vector.tensor_scalar(halfw[:, :], bc37[:, 2:3], 0.5, None, OP.mult)
    nc.vector.tensor_scalar(halfh[:, :], bc37[:, 3:4], 0.5, None, OP.mult)

    def clipped(dst, src_col, half, op, sxy):
        t = sb.tile([128, 1], F32, tag=dst + "_t")
        nc.vector.tensor_tensor(t[:, :], bc37[:, src_col:src_col + 1],
                                half[:, :], op)
        nc.vector.tensor_scalar(t[:, :], t[:, :], 0.0, float(IMGSZ - 1),
                                OP.max, OP.min)
        o = sb.tile([128, 1], F32, tag=dst)
        nc.vector.tensor_scalar(o[:, :], t[:, :], sxy, None, OP.mult)
        return o

    SX, SY = W0 / IMGSZ, H0 / IMGSZ
    fb0 = clipped("fb0", 0, halfw, OP.subtract, SX)
    fb1 = clipped("fb1", 1, halfh, OP.subtract, SY)
    fb2 = clipped("fb2", 0, halfw, OP.add, SX)
    fb3 = clipped("fb3", 1, halfh, OP.add, SY)

    # window base: m = clamp(round(fb0/6 - 1.5), 0, 554); c0 = 6m
    c0m = sb.tile([128, 1], F32, tag="c0m")
    nc.vector.tensor_scalar(c0m[:, :], fb0[:, :], 1.0 / 6.0, 1.5, OP.mult,
                            OP.subtract)
    nc.vector.tensor_scalar(c0m[:, :], c0m[:, :], 0.0, 554.0, OP.max, OP.min)
    m_i = sb.tile([1, 1], I32, tag="m_i")
    nc.vector.tensor_copy(m_i[0:1, :], c0m[0:1, :])
    c0_i = sb.tile([1, 1], I32, tag="c0_i")
    nc.vector.tensor_scalar(c0_i[:, :], m_i[:, :], 6, None, OP.mult)
    c0f1 = sb.tile([1, 1], F32, tag="c0f1")
    nc.vector.tensor_copy(c0f1[0:1, :], c0_i[0:1, :])
    psC = ps.tile([128, 1], F32, tag="ps")
    nc.tensor.matmul(psC[:, :], ones1[:, :], c0f1[:, :], start=True, stop=True)
    c0col = sb.tile([128, 1], F32, tag="c0col")
    nc.scalar.copy(c0col[:, :], psC[:, :])

    # meta output for host fallback check: [a, fb0..3, c0]
    metas = sb.tile([1, 8], F32, tag="metas")
    nc.vector.tensor_copy(metas[0:1, 0:1], a_f[0:1, :])
    nc.vector.tensor_copy(metas[0:1, 1:2], fb0[0:1, :])
    nc.vector.tensor_copy(metas[0:1, 2:3], fb1[0:1, :])
    nc.vector.tensor_copy(metas[0:1, 3:4], fb2[0:1, :])
    nc.vector.tensor_copy(metas[0:1, 4:5], fb3[0:1, :])
    nc.vector.tensor_copy(metas[0:1, 5:6], c0f1[0:1, :])
    nc.vector.tensor_copy(metas[0:1, 6:7], gsc[0:1, :])
    nc.vector.tensor_copy(metas[0:1, 7:8], fsc[0:1, :])
    nc.sync.dma_start(meta_d.ap(), metas[:, :])

    if stage <= 2:
        ctx.close()
        return

    # ---------------- stage M: mask pipeline ----------------
    protos = sb.tile([32, MROWS * 160], F32, tag="protos")
    nc.sync.dma_start(protos[:, :], protos_d.ap())
    ahst = sb.tile([MROWS, SROWS], F32, tag="ahst")
    nc.sync.dma_start(ahst[:, :], ahst_d.ap())
    awt_a = sb.tile([128, IMGSZ], F32, tag="awt_a")
    awt_b = sb.tile([32, IMGSZ], F32, tag="awt_b")
    nc.sync.dma_start(awt_a[:, :], awt_d.ap()[0:128, :])
    nc.sync.dma_start(awt_b[:, :], awt_d.ap()[128:160, :])
    r82 = sb.tile([SROWS, ROWS], F32, tag="r82")
    nc.sync.dma_start(r82[:, :], r82_d.ap())
    riota = sb.tile([128, 3], F32, tag="riota")
    nc.sync.dma_start(riota[:, :], riota_d.ap())

    m160f = sb.tile([1, MROWS * 160], F32, tag="m160f")
    HALF = MROWS * 160 // 2                            # 1920 = 4 banks
    for h in range(2):
        psM = ps.tile([1, HALF], F32, tag="psM", name=f"psM{rep}_{h}")
        for i in range(4):
            n0, n1 = 512 * i, min(512 * (i + 1), HALF)
            nc.tensor.matmul(psM[0:1, n0:n1], coefT,
                             protos[:, HALF * h + n0:HALF * h + n1],
                             start=True, stop=True)
        nc.scalar.copy(m160f[0:1, HALF * h:HALF * (h + 1)], psM[:, :])
    m160r = sb.tile([MROWS, 160], F32, tag="m160r")
    nc.sync.dma_start(
        m160r[:, :],
        m160f[:, :].rearrange("q (h w) -> (q h) w", w=160))

    # step1: P1T[w160, h82] = sum_h m160r[h, w] * ahst[h, j]
    psP1 = ps.tile([128, SROWS], F32, tag="ps")
    psP2 = ps.tile([32, SROWS], F32, tag="ps")
    nc.tensor.matmul(psP1[:, :], m160r[:, 0:128], ahst[:, :], start=True, stop=True)
    nc.tensor.matmul(psP2[:, :], m160r[:, 128:160], ahst[:, :], start=True, stop=True)
    p1 = sb.tile([128, SROWS], F32, tag="p1")
    p2 = sb.tile([32, SROWS], F32, tag="p2")
    nc.scalar.copy(p1[:, :], psP1[:, :])
    nc.scalar.copy(p2[:, :], psP2[:, :])

    # step2: m640rows[h82, j640] = sum_w P1T[w, h] * AwT[w, j]
    psQ = ps.tile([SROWS, IMGSZ], F32, tag="ps")
    for n0, n1 in ((0, 512), (512, 640)):
        nc.tensor.matmul(psQ[:, n0:n1], p1[:, :], awt_a[:, n0:n1],
                         start=True, stop=False)
        nc.tensor.matmul(psQ[:, n0:n1], p2[:, :], awt_b[:, n0:n1],
                         start=False, stop=True)

    s_pad = sb.tile([SROWS, 642], F32, tag="s_pad")
    act(s_pad[:, 1:641], psQ[:, :], AF.Sigmoid)
    nc.vector.tensor_copy(s_pad[:, 0:1], s_pad[:, 1:2])
    nc.vector.tensor_copy(s_pad[:, 641:642], s_pad[:, 640:641])

    if stage <= 3:
        ctx.close()
        return

    # dynamic windows
    s_win = sb.tile([SROWS, SWIN], F32, tag="s_win")
    vww = sb.tile([SWIN, WWIN], F32, tag="vww")
    xw = [sb.tile([128, 3 * WWIN], F32, tag=f"xw{t}", name=f"xw{rep}_{t}")
          for t in range(3)]
    RT = [128, 128, ROWS - 256]
    with nc.gpsimd.register(f"mo{rep}") as mreg, \
            nc.gpsimd.register(f"co_{rep}") as creg:
        nc.gpsimd.reg_load(mreg, m_i[0:1, 0:1])
        nc.gpsimd.reg_load(creg, c0_i[0:1, 0:1])
        mo = nc.gpsimd.snap(mreg, min_val=0, max_val=554)
        co = nc.gpsimd.snap(creg, min_val=0, max_val=W0 - WWIN)
        nc.gpsimd.dma_start(s_win[:, :], s_pad[:, bass.ds(mo, SWIN)])
        nc.gpsimd.dma_start(vww[:, :],
                            vwpad_d.ap()[bass.ds(mo, SWIN), bass.ds(co, WWIN)])
        xt = x_d.ap().transpose([1, 0, 2])      # [270, 3, 3840]
        for t in range(3):
            nc.gpsimd.dma_start(
                xw[t][0:RT[t], :].rearrange("p (c w) -> p c w", c=3),
                xt[128 * t:128 * t + RT[t], :, bass.ds(co, WWIN)])

    if stage <= 4:
        ctx.close()
        return

    # uT[i88, m270] = sum_p s_win[p, i] * r82[p, m]
    psU = ps.tile([SWIN, ROWS], F32, tag="ps")
    nc.tensor.matmul(psU[:, :], s_win[:, :], r82[:, :], start=True, stop=True)
    uT = sb.tile([SWIN, ROWS], F32, tag="uT")
    nc.scalar.copy(uT[:, :], psU[:, :])

    if stage <= 5:
        ctx.close()
        return

    # ---------------- stage O: threshold + rect + multiply ----------------
    xio = sb.tile([128, WWIN], I32, tag="xio")
    nc.gpsimd.iota(xio[:, :], [[1, WWIN]], channel_multiplier=0)
    xiof = sb.tile([128, WWIN], F32, tag="xiof")
    nc.vector.tensor_copy(xiof[:, :], xio[:, :])
    xcol = sb.tile([128, WWIN], F32, tag="xcol")
    nc.vector.tensor_scalar(xcol[:, :], xiof[:, :], c0col[:, :], None, OP.add)
    cma = sb.tile([128, WWIN], F32, tag="cma")
    nc.vector.tensor_scalar(cma[:, :], xcol[:, :], fb0[:, :], 255.0,
                            OP.is_ge, OP.mult)
    cmb = sb.tile([128, WWIN], F32, tag="cmb")
    nc.vector.tensor_scalar(cmb[:, :], xcol[:, :], fb2[:, :], None, OP.is_lt)
    cm255 = sb.tile([128, WWIN], F32, tag="cm255")
    nc.vector.tensor_tensor(cm255[:, :], cma[:, :], cmb[:, :], OP.mult)

    out_t = out_d.ap().transpose([1, 0, 2])         # [270, 3, 3840]
    for t in range(3):
        rt = RT[t]
        psW = ps.tile([128, WWIN], F32, tag="ps", name=f"psW{rep}_{t}")
        nc.tensor.matmul(psW[0:rt, 0:512], uT[:, 128 * t:128 * t + rt],
                         vww[:, 0:512], start=True, stop=True)
        nc.tensor.matmul(psW[0:rt, 512:WWIN], uT[:, 128 * t:128 * t + rt],
                         vww[:, 512:WWIN], start=True, stop=True)
        sgn = sb.tile([128, WWIN], F32, tag=f"sgn{t}")
        act(sgn[0:rt, :], psW[0:rt, :], AF.Sign, bias=-MASK_THR)
        rma = sb.tile([128, 1], F32, tag=f"rma{t}")
        nc.vector.tensor_scalar(rma[:, :], riota[:, t:t + 1], fb1[:, :], None,
                                OP.is_ge)
        rmb = sb.tile([128, 1], F32, tag=f"rmb{t}")
        nc.vector.tensor_scalar(rmb[:, :], riota[:, t:t + 1], fb3[:, :], None,
                                OP.is_lt)
        rm = sb.tile([128, 1], F32, tag=f"rm{t}")
        nc.vector.tensor_tensor(rm[:, :], rma[:, :], rmb[:, :], OP.mult)
        bm = sb.tile([128, WWIN], F32, tag=f"bm{t}")
        nc.vector.tensor_scalar(bm[0:rt, :], sgn[0:rt, :], 0.0, rm[0:rt, :],
                                OP.max, OP.mult)
        bm3 = sb.tile([128, WWIN], F32, tag=f"bm3{t}")
        nc.vector.tensor_tensor(bm3[0:rt, :], bm[0:rt, :], cm255[0:rt, :],
                                OP.mult)
        res = sb.tile([128, 3 * WWIN], F32, tag=f"res{t}")
        for ch in range(3):
            nc.vector.tensor_tensor(
                res[0:rt, WWIN * ch:WWIN * (ch + 1)],
                xw[t][0:rt, WWIN * ch:WWIN * (ch + 1)], bm3[0:rt, :], OP.mult)
        with nc.gpsimd.register(f"co{rep}_{t}") as creg:
            nc.gpsimd.reg_load(creg, c0_i[0:1, 0:1])
            co = nc.gpsimd.snap(creg, min_val=0, max_val=W0 - WWIN)
            nc.gpsimd.dma_start(
                out_t[128 * t:128 * t + rt, :, bass.ds(co, WWIN)],
                res[0:rt, :].rearrange("p (c w) -> p c w", c=3))

    ctx.close()


# ---------------------------------------------------------------------------
# host orchestration
# ---------------------------------------------------------------------------

_NC_CACHE = None


def _get_nc():
    global _NC_CACHE
    if _NC_CACHE is None:
        _NC_CACHE = _build_nc()
    return _NC_CACHE


def _numpy_fallback(x_raw, pred, proto):
    """Exact slow-path reference (only used if the rect exceeds the device
    window, which cannot happen for in-distribution inputs)."""
    p = pred[0]
    boxes, cls, coef = p[:, :4], p[:, 4], p[:, 5:]
    s1 = np.maximum(1.0 / (1.0 + np.exp(-cls)) - 0.5, 0) + np.float32(0.001)
    mk = np.abs(coef).sum(-1)
    f = np.float32(640.0 if boxes.max() <= 1.2 else 1.0)
    dxdy = np.abs(boxes[:, :2] * f - 320.0) / 320.0
    cw = np.maximum(1.0 - 0.5 * (dxdy[:, 0] + dxdy[:, 1]), 0.0)
    a = int(np.argmax(s1 * mk * (0.5 + 0.5 * cw)))
    fcoef = coef[a]
    cx, cy, w, h = boxes[a]
    xyxy = np.clip(np.array([cx - w / 2, cy - h / 2, cx + w / 2, cy + h / 2],
                            np.float32), 0.0, IMGSZ - 1)
    fb = xyxy * np.array([W0 / IMGSZ, H0 / IMGSZ, W0 / IMGSZ, H0 / IMGSZ],
                         np.float32)
    Ah = _weight_mat(160, IMGSZ)
    Aw = _weight_mat(160, IMGSZ)
    Vh = _weight_mat(IMGSZ, H0)
    Vw = _weight_mat(IMGSZ, W0)
    m160 = (fcoef @ proto[0].reshape(32, -1)).reshape(160, 160)
    m640 = Ah.T @ m160 @ Aw
    s640 = 1.0 / (1.0 + np.exp(-m640))
    m_orig = (Vh.T @ s640 @ Vw).astype(np.float32)
    ys = np.arange(H0, dtype=np.float32)[:, None]
    xs = np.arange(W0, dtype=np.float32)[None, :]
    rect = (xs >= fb[0]) & (xs < fb[2]) & (ys >= fb[1]) & (ys < fb[3])
    bm = ((m_orig > MASK_THR) & rect).astype(np.float32)
    return (np.clip(x_raw * 255.0, 0.0, 255.0) * bm[None, None]).astype(np.float32)


def kernel(x_raw, pred, proto):
    x_raw = np.ascontiguousarray(np.asarray(x_raw, dtype=np.float32))
    pred = np.ascontiguousarray(np.asarray(pred, dtype=np.float32))
    proto = np.ascontiguousarray(np.asarray(proto, dtype=np.float32))

    nc = _get_nc()
    shared, percore = _host_consts()
    pred2 = np.ascontiguousarray(pred[0])           # [8400, 37]
    proto2 = proto[0]                               # [32, 160, 160]

    in_maps = []
    for c in range(N_CORES):
        pc = percore[c]
        ha = pc["ha"]
        protos_sl = np.ascontiguousarray(
            proto2[:, ha:ha + MROWS, :].reshape(32, MROWS * 160))
        in_maps.append({
            "pred": pred2,
            "xs": np.ascontiguousarray(x_raw[0, :, ROWS * c:ROWS * (c + 1), :]),
            "protos": protos_sl,
            "ahst": pc["ahst"],
            "awt": shared["awt"],
            "r82": pc["r82"],
            "vwpad": shared["vwpad"],
            "ones1": shared["ones1"],
            "id1": shared["id1"],
            "riota": pc["riota"],
        })

    res = bass_utils.run_bass_kernel_spmd(nc, in_maps,
                                          core_ids=list(range(N_CORES)))

    meta = res.results[0]["meta"][0]
    fb0, fb1, fb2, fb3, c0 = meta[1], meta[2], meta[3], meta[4], meta[5]
    # fallback if the rect column span is not covered by the device window
    covered = (fb2 <= fb0) or (
        (c0 <= np.floor(fb0)) and (np.ceil(fb2) <= c0 + WWIN))
    if not covered:
        return _numpy_fallback(x_raw, pred, proto)

    out = np.concatenate([res.results[c]["out"] for c in range(N_CORES)],
                         axis=1)          # [3, 2160, 3840]
    return out[None]


if __name__ == "__main__":
    import jax
    with jax.default_device(jax.devices("cpu")[0]):
        import reference as R
        inputs = R.setup_inputs()
        inputs = {k: np.asarray(v) for k, v in inputs.items()}
    out = kernel(**inputs)
    ref = np.load("/tmp/ref_out.npy")
    print("absmax:", np.abs(out - ref).max())


# revision 23
# speedup vs baseline: 230.2042x; 1.2424x over previous
"""Trainium2 Bass kernel for nn_End2EndTongueROI_Dynamic_NMS.

Key algebraic facts used (verified against the reference):
  - Greedy NMS always keeps the top-scored box first and fi=argmax(keep)=0,
    so the whole top-k/NMS tail reduces to argmax(score) over 8400 anchors.
  - score's /max(maskness) normalization and /32 mean are positive scalings
    shared by all anchors -> argmax-invariant -> dropped on device.
  - Both resizes are linear: expressed as matmuls with exact f32 weight
    matrices replicated from jax.image.resize's compute_weight_mat.
  - The final rect is data-dependent but narrow (box w,h ~ U[0,1) in the
    reference's encoding, and the rect is built from *unscaled* xyxy), so
    each core computes its 270-row shard restricted to a dynamic 128-row x
    516-column window that covers the rect. Everything outside the window is
    exactly 0 in the reference output and the PJRT path pre-zeroes/donates
    output buffers (documented contract: "kernels that don't write every
    element rely on that"), so only the window is written. A host-side check
    falls back to exact numpy if the rect ever exceeds the window
    (impossible for in-distribution inputs).
  - The mask pipeline is computed only over the window's dependency cone:
    24 of 160 proto rows (H), a dynamic 28-wide column slice (W), a 2-matmul
    coef matvec, one matmul per resize leg, at exact jax f32 weights.

Sharding: H0=2160 rows split 8 x 270. Score fusion + argmax tail is tiny and
fully replicated per core (no collectives needed).
"""
import numpy as np

import concourse.bacc as bacc
import concourse.bass as bass
import concourse.mybir as mybir
import concourse.tile as tile
from concourse import bass_isa, bass_utils

F32 = mybir.dt.float32
I32 = mybir.dt.int32
U32 = mybir.dt.uint32

N_CORES = 8
H0, W0 = 2160, 3840
IMGSZ = 640
MASK_THR = 0.72
NANCH, NC_COL = 8400, 37
ROWS = H0 // N_CORES          # 270 rows per core
SROWS = 82                    # s640 row window per core
MROWS = 24                    # m160 row window per core (padded)
WWIN = 516                    # output column window (6*86)
SWIN = 88                     # s-column window feeding WWIN
WW160 = 28                    # m160 column window feeding SWIN
RWIN = 128                    # output row window (one partition tile)
NPP = 66                      # anchors per partition (66*128 = 8448 >= 8400)
# sentinel for the argmin-over-winners trick; power of two > NANCH so that
# af - BIG and +BIG round-trip exactly in f32 (af < 2^14, ulp stays <= 1)
BIG = 16384.0


# ---------------------------------------------------------------------------
# host-side resize weights (exact replica of jax.image.resize bilinear)
# ---------------------------------------------------------------------------

def _weight_mat(in_size, out_size):
    dt = np.float32
    scale = dt(out_size / in_size)
    inv_scale = dt(1.0) / scale
    sample_f = (np.arange(out_size, dtype=dt) + dt(0.5)) * inv_scale - dt(0.5)
    x = np.abs(sample_f[None, :] - np.arange(in_size, dtype=dt)[:, None])
    w = np.maximum(dt(0), dt(1) - x).astype(dt)
    tot = w.sum(axis=0, keepdims=True).astype(dt)
    w = np.where(np.abs(tot) > 1000.0 * np.finfo(np.float32).eps,
                 w / np.where(tot != 0, tot, 1), 0).astype(dt)
    ok = (sample_f >= -0.5) & (sample_f <= in_size - 0.5)
    return np.where(ok[None, :], w, 0).astype(dt)


def _host_consts():
    """Constant tensors. Returns (shared, percore_list)."""
    Ah = _weight_mat(160, IMGSZ)      # [160, 640]
    Aw = _weight_mat(160, IMGSZ)      # [160, 640]
    Vh = _weight_mat(IMGSZ, H0)       # [640, 2160]
    Vw = _weight_mat(IMGSZ, W0)       # [640, 3840]

    # vwpad row i+1 = Vw row i (s-col i); zero guard rows at both ends so the
    # dynamic [SWIN, WWIN] slice at row m covers s-cols [m-1, m+86] with the
    # out-of-range ends contributing exactly zero.
    vwpad = np.zeros((642, W0), np.float32)
    vwpad[1:641] = Vw
    # AwT with the same one-column zero guard on both sides: awtp[w, j+1] =
    # Aw[w, j].  The dynamic [WW160, SWIN] slice at (ww, m) then aligns
    # column-for-column with the vwpad slice rows.
    awtp = np.zeros((160, 642), np.float32)
    awtp[:, 1:641] = Aw

    ones1 = np.ones((1, 128), np.float32)
    id1 = np.ones((1, 1), np.float32)
    xiota = np.broadcast_to(np.arange(WWIN, dtype=np.float32),
                            (128, WWIN)).copy()
    pio1 = np.arange(128, dtype=np.float32).reshape(128, 1).copy()
    pio66 = (np.arange(128, dtype=np.float32) * NPP).reshape(128, 1).copy()

    percore = []
    for c in range(N_CORES):
        r0 = ROWS * c
        vh_sl = Vh[:, r0:r0 + ROWS]
        nz = np.where(vh_sl.any(axis=1))[0]
        ra = min(int(nz.min()), IMGSZ - SROWS)
        r82 = np.ascontiguousarray(vh_sl[ra:ra + SROWS, :])     # [82, 270]

        ah_sl = Ah[:, ra:ra + SROWS]                            # [160, 82]
        nzh = np.where(ah_sl.any(axis=1))[0]
        ha = min(int(nzh.min()), 160 - MROWS)
        ahst = np.ascontiguousarray(ah_sl[ha:ha + MROWS, :])    # [24, 82]

        r0c = np.full((1, 1), float(r0), np.float32)
        percore.append(dict(ra=ra, ha=ha, r82=r82, ahst=ahst, r0c=r0c))

    shared = dict(awtp=awtp, vwpad=vwpad, ones1=ones1, id1=id1, xiota=xiota,
                  pio1=pio1, pio66=pio66)
    return shared, percore


# ---------------------------------------------------------------------------
# device program (identical for all cores; per-core data comes via inputs)
# ---------------------------------------------------------------------------

def _build_nc(stage=99, reps=1, loop_n=0):
    nc = bacc.Bacc("TRN2", target_bir_lowering=False, debug=False,
                   enable_asserts=False, num_devices=N_CORES)

    d = {}
    d["pred"] = nc.dram_tensor("pred", [NANCH, NC_COL], F32, kind="ExternalInput")
    d["xs"] = nc.dram_tensor("xs", [3, ROWS, W0], F32, kind="ExternalInput")
    d["protos"] = nc.dram_tensor("protos", [32, MROWS * 160], F32, kind="ExternalInput")
    d["ahst"] = nc.dram_tensor("ahst", [MROWS, SROWS], F32, kind="ExternalInput")
    d["awtp"] = nc.dram_tensor("awtp", [160, 642], F32, kind="ExternalInput")
    d["r82"] = nc.dram_tensor("r82", [SROWS, ROWS], F32, kind="ExternalInput")
    d["vwpad"] = nc.dram_tensor("vwpad", [642, W0], F32, kind="ExternalInput")
    d["ones1"] = nc.dram_tensor("ones1", [1, 128], F32, kind="ExternalInput")
    d["id1"] = nc.dram_tensor("id1", [1, 1], F32, kind="ExternalInput")
    d["xiota"] = nc.dram_tensor("xiota", [128, WWIN], F32, kind="ExternalInput")
    d["pio1"] = nc.dram_tensor("pio1", [128, 1], F32, kind="ExternalInput")
    d["pio66"] = nc.dram_tensor("pio66", [128, 1], F32, kind="ExternalInput")
    d["r0c"] = nc.dram_tensor("r0c", [1, 1], F32, kind="ExternalInput")

    d["out"] = nc.dram_tensor("out", [3, ROWS, W0], F32, kind="ExternalOutput")
    d["meta"] = nc.dram_tensor("meta", [1, 8], F32, kind="ExternalOutput")

    with tile.TileContext(nc) as tc:
        if loop_n:
            with tc.For_i(0, loop_n, 1):
                _program(nc, tc, d, stage, 0)
        else:
            for rep in range(reps):
                _program(nc, tc, d, stage, rep)
    nc.compile()
    return nc


def _program(nc, tc, d, stage=99, rep=0):
    AF = mybir.ActivationFunctionType
    OP = mybir.AluOpType
    AX = mybir.AxisListType
    import contextlib
    ctx = contextlib.ExitStack()

    sb = ctx.enter_context(tc.tile_pool(name="sb", bufs=1))
    ps = ctx.enter_context(tc.tile_pool(name="ps", bufs=2,
                                        space=bass.MemorySpace.PSUM))

    _bias_cache = {}

    def cbias(val):
        if val not in _bias_cache:
            t = sb.tile([128, 1], F32, tag=f"cb{len(_bias_cache)}",
                        name=f"cb{rep}_{len(_bias_cache)}")
            nc.vector.memset(t[:, :], val)
            _bias_cache[val] = t
        return _bias_cache[val]

    def act(out_ap, in_ap, func, bias=0.0, scale=1.0):
        nparts = in_ap.shape[0]
        nc.scalar.activation(out_ap, in_ap, func,
                             bias=cbias(float(bias))[0:nparts, :],
                             scale=scale)

    def ts(out_ap, in_ap, s1, s2, op0, op1=None):
        nc.vector.tensor_scalar(out_ap, in_ap, s1, s2, op0,
                                *([] if op1 is None else [op1]))

    def tt(out_ap, a_ap, b_ap, op):
        nc.vector.tensor_tensor(out_ap, a_ap, b_ap, op)

    def tile1(tag, shape=(128, 1), dtype=F32):
        return sb.tile(list(shape), dtype, tag=tag, name=f"{tag}_{rep}")

    # small consts
    ones1 = tile1("ones1", (1, 128))
    nc.sync.dma_start(ones1[:, :], d["ones1"].ap())
    id1 = tile1("id1", (1, 1))
    nc.sync.dma_start(id1[:, :], d["id1"].ap())
    pio1 = tile1("pio1")
    nc.sync.dma_start(pio1[:, :], d["pio1"].ap())
    pio66 = tile1("pio66")
    nc.sync.dma_start(pio66[:, :], d["pio66"].ap())
    r0c = tile1("r0c", (1, 1))
    nc.sync.dma_start(r0c[:, :], d["r0c"].ap())

    def bcast_col(name, src11):
        """[1,1] f32 -> [128,1] via K=1 matmul + copy."""
        p = ps.tile([128, 1], F32, tag="ps", name=f"psb_{name}_{rep}")
        nc.tensor.matmul(p[:, :], ones1[:, :], src11, start=True, stop=True)
        o = tile1(name)
        nc.scalar.copy(o[:, :], p[:, :])
        return o

    # ---------------- stage S: score fusion + argmax ----------------
    # anchor a = p*66 + n; partition 127 cols >= 18 are uninitialized pad
    # (8400 = 127*66 + 18), zeroed via a DMA from vwpad's zero row (compute
    # engines cannot target a lone partition 127, and a memset+overlapping-DMA
    # combination wedges the HW even though CoreSim accepts it).
    P2 = tile1("P2", (128, NPP * NC_COL))
    nc.sync.dma_start(P2[127:128, 18 * NC_COL:],
                      d["vwpad"].ap()[0:1, 0:(NPP - 18) * NC_COL])
    nc.sync.dma_start(
        P2[0:127, :],
        d["pred"].ap()[0:127 * NPP, :].rearrange("(p n) c -> p (n c)", n=NPP))
    nc.sync.dma_start(
        P2[127:128, 0:18 * NC_COL],
        d["pred"].ap()[127 * NPP:NANCH, :].rearrange("(p n) c -> p (n c)", p=1))

    P3 = P2[:, :].rearrange("p (n c) -> p n c", c=NC_COL)   # [128, 66, 37]

    sg = tile1("sg", (128, NPP))
    act(sg[:, :], P3[:, :, 4], AF.Sigmoid)
    s2 = tile1("s2", (128, NPP))
    ts(s2[:, :], sg[:, :], -0.5, 0.0, OP.add, OP.max)       # relu(sig-0.5)
    ts(s2[:, :], s2[:, :], 0.001, None, OP.add)

    mk = tile1("mk", (128, NPP))
    nc.vector.tensor_reduce(mk[:, :], P3[:, :, 5:NC_COL], AX.X, OP.add,
                            apply_absolute_value=True)

    gm1 = tile1("gm1")
    nc.vector.tensor_reduce(gm1[:, :], P3[:, :, 0:4], AX.XY, OP.max)
    gmax = tile1("gmax")
    nc.gpsimd.partition_all_reduce(gmax[:, :], gm1[:, :], 128,
                                   bass_isa.ReduceOp.max)
    fsc = tile1("fsc")
    ts(fsc[:, :], gmax[:, :], 1.2, 639.0, OP.is_le, OP.mult)
    ts(fsc[:, :], fsc[:, :], 1.0, None, OP.add)

    dxa = tile1("dxa", (128, NPP))
    dya = tile1("dya", (128, NPP))
    act(dxa[:, :], P3[:, :, 0], AF.Abs, bias=-320.0, scale=fsc[:, :])
    act(dya[:, :], P3[:, :, 1], AF.Abs, bias=-320.0, scale=fsc[:, :])
    uxy = tile1("uxy", (128, NPP))
    tt(uxy[:, :], dxa[:, :], dya[:, :], OP.add)
    cwf = tile1("cwf", (128, NPP))
    ts(cwf[:, :], uxy[:, :], -1.0 / 640.0, 1.0, OP.mult, OP.add)
    ts(cwf[:, :], cwf[:, :], 0.0, 0.5, OP.max, OP.mult)
    ts(cwf[:, :], cwf[:, :], 0.5, None, OP.add)

    score = tile1("score", (128, NPP))
    tt(score[:, :], s2[:, :], mk[:, :], OP.mult)
    tt(score[:, :], score[:, :], cwf[:, :], OP.mult)

    vmax8 = tile1("vmax8", (128, 8))
    vidx8 = tile1("vidx8", (128, 8), U32)
    nc.vector.max_with_indices(vmax8[:, :], vidx8[:, :], score[:, :])

    gsc = tile1("gsc")
    nc.gpsimd.partition_all_reduce(gsc[:, :], vmax8[:, 0:1], 128,
                                   bass_isa.ReduceOp.max)

    af = tile1("af")
    nc.vector.tensor_copy(af[:, :], vidx8[:, 0:1])
    ts(af[:, :], af[:, :], pio66[:, :], -BIG, OP.add, OP.add)
    wm = tile1("wm")
    ts(wm[:, :], vmax8[:, 0:1], gsc[:, :], None, OP.is_ge)
    cand = tile1("cand")
    tt(cand[:, :], af[:, :], wm[:, :], OP.mult)
    ts(cand[:, :], cand[:, :], BIG, -1.0, OP.add, OP.mult)
    mxn = tile1("mxn")
    nc.gpsimd.partition_all_reduce(mxn[:, :], cand[:, :], 128,
                                   bass_isa.ReduceOp.max)
    a_f = tile1("a_f")
    ts(a_f[:, :], mxn[:, :], -1.0, None, OP.mult)
    a_i = tile1("a_i", (1, 1), I32)
    nc.vector.tensor_copy(a_i[0:1, :], a_f[0:1, :])

    if stage <= 1:
        metas = tile1("metas", (1, 8))
        nc.vector.memset(metas[:, :], 0.0)
        nc.vector.tensor_copy(metas[0:1, 0:1], a_f[0:1, :])
        nc.sync.dma_start(d["meta"].ap(), metas[:, :])
        ctx.close()
        return

    # ---------------- stage G: gather winner row; box -> windows ----------
    row1 = tile1("row1", (1, NC_COL))
    with nc.gpsimd.register(f"aoff{rep}") as areg:
        nc.gpsimd.reg_load(areg, a_i[0:1, 0:1])
        aoff = nc.gpsimd.snap(areg, min_val=0, max_val=NANCH - 1)
        nc.gpsimd.dma_start(row1[:, :], d["pred"].ap()[bass.ds(aoff, 1), :])

    psB = ps.tile([128, NC_COL], F32, tag="ps", name=f"psB{rep}")
    nc.tensor.matmul(psB[:, :], ones1[:, :], row1[:, :], start=True, stop=True)
    bc37 = tile1("bc37", (128, NC_COL))
    nc.scalar.copy(bc37[:, :], psB[:, :])

    psT = ps.tile([32, 1], F32, tag="ps", name=f"psT{rep}")
    nc.tensor.transpose(psT[:, :], row1[:, 5:NC_COL], id1[:, :])
    coefT = tile1("coefT", (32, 1))
    nc.scalar.copy(coefT[:, :], psT[:, :])

    # box -> fb (full-res rect bounds), [128,1] broadcast columns
    halfw = tile1("halfw")
    halfh = tile1("halfh")
    ts(halfw[:, :], bc37[:, 2:3], 0.5, None, OP.mult)
    ts(halfh[:, :], bc37[:, 3:4], 0.5, None, OP.mult)

    def clipped(dst, src_col, half, op, sxy):
        t = tile1(dst + "_t")
        tt(t[:, :], bc37[:, src_col:src_col + 1], half[:, :], op)
        ts(t[:, :], t[:, :], 0.0, float(IMGSZ - 1), OP.max, OP.min)
        o = tile1(dst)
        ts(o[:, :], t[:, :], sxy, None, OP.mult)
        return o

    SX, SY = W0 / IMGSZ, H0 / IMGSZ
    fb0 = clipped("fb0", 0, halfw, OP.subtract, SX)
    fb1 = clipped("fb1", 1, halfh, OP.subtract, SY)
    fb2 = clipped("fb2", 0, halfw, OP.add, SX)
    fb3 = clipped("fb3", 1, halfh, OP.add, SY)

    # column window: m = clamp(round(fb0/6 - 1.5), 0, 554); c0 = 6m
    c0m = tile1("c0m")
    ts(c0m[:, :], fb0[:, :], 1.0 / 6.0, 1.5, OP.mult, OP.subtract)
    ts(c0m[:, :], c0m[:, :], 0.0, 554.0, OP.max, OP.min)
    m_i = tile1("m_i", (1, 1), I32)
    nc.vector.tensor_copy(m_i[0:1, :], c0m[0:1, :])
    c0_i = tile1("c0_i", (1, 1), I32)
    ts(c0_i[:, :], m_i[:, :], 6, None, OP.mult)
    c0f1 = tile1("c0f1", (1, 1))
    nc.vector.tensor_copy(c0f1[0:1, :], c0_i[0:1, :])
    c0col = bcast_col("c0col", c0f1[:, :])

    # m160 column window: ww = clamp(floor(m/4) - 1, 0, 132) via
    # round(m/4 - 1.375) (fractions of m/4 are k/4 so the .375 offset rounds
    # to exactly floor(m/4) - 1)
    m_f = tile1("m_f", (1, 1))
    nc.vector.tensor_copy(m_f[0:1, :], m_i[0:1, :])
    wwf = tile1("wwf", (1, 1))
    ts(wwf[:, :], m_f[:, :], 0.25, 1.375, OP.mult, OP.subtract)
    ts(wwf[:, :], wwf[:, :], 0.0, float(160 - WW160), OP.max, OP.min)
    ww_i = tile1("ww_i", (1, 1), I32)
    nc.vector.tensor_copy(ww_i[0:1, :], wwf[0:1, :])

    # row window: rw = clamp(round(fb1 - r0 - 1.5), 0, 142)
    rwt = tile1("rwt", (1, 1))
    tt(rwt[0:1, :], fb1[0:1, :], r0c[:, :], OP.subtract)
    ts(rwt[:, :], rwt[:, :], 1.5, None, OP.subtract)
    ts(rwt[:, :], rwt[:, :], 0.0, float(ROWS - RWIN), OP.max, OP.min)
    rw_i = tile1("rw_i", (1, 1), I32)
    nc.vector.tensor_copy(rw_i[0:1, :], rwt[0:1, :])
    rw_f = tile1("rw_f", (1, 1))
    nc.vector.tensor_copy(rw_f[0:1, :], rw_i[0:1, :])
    rbase1 = tile1("rbase1", (1, 1))
    tt(rbase1[0:1, :], rw_f[0:1, :], r0c[:, :], OP.add)
    rbase = bcast_col("rbase", rbase1[:, :])
    riog = tile1("riog")                       # global row index per partition
    tt(riog[:, :], pio1[:, :], rbase[:, :], OP.add)

    # meta output for the host coverage check: [a, fb0..3, c0, rw, fsc]
    metas = tile1("metas", (1, 8))
    nc.vector.tensor_copy(metas[0:1, 0:1], a_f[0:1, :])
    nc.vector.tensor_copy(metas[0:1, 1:2], fb0[0:1, :])
    nc.vector.tensor_copy(metas[0:1, 2:3], fb1[0:1, :])
    nc.vector.tensor_copy(metas[0:1, 3:4], fb2[0:1, :])
    nc.vector.tensor_copy(metas[0:1, 4:5], fb3[0:1, :])
    nc.vector.tensor_copy(metas[0:1, 5:6], c0f1[0:1, :])
    nc.vector.tensor_copy(metas[0:1, 6:7], rw_f[0:1, :])
    nc.vector.tensor_copy(metas[0:1, 7:8], fsc[0:1, :])
    nc.sync.dma_start(d["meta"].ap(), metas[:, :])

    if stage <= 2:
        ctx.close()
        return

    # ---------------- stage M: windowed mask pipeline ----------------
    ahst = tile1("ahst", (MROWS, SROWS))
    nc.sync.dma_start(ahst[:, :], d["ahst"].ap())
    r82 = tile1("r82", (SROWS, ROWS))
    nc.sync.dma_start(r82[:, :], d["r82"].ap())

    protosw = tile1("protosw", (32, MROWS * WW160))
    awW = tile1("awW", (WW160, SWIN))
    vww = tile1("vww", (SWIN, WWIN))
    xw = tile1("xw", (128, 3 * WWIN))
    with nc.gpsimd.register(f"mo{rep}") as mreg, \
            nc.gpsimd.register(f"wo{rep}") as wreg, \
            nc.gpsimd.register(f"co_{rep}") as creg, \
            nc.gpsimd.register(f"ro{rep}") as rreg:
        nc.gpsimd.reg_load(mreg, m_i[0:1, 0:1])
        nc.gpsimd.reg_load(wreg, ww_i[0:1, 0:1])
        nc.gpsimd.reg_load(creg, c0_i[0:1, 0:1])
        nc.gpsimd.reg_load(rreg, rw_i[0:1, 0:1])
        mo = nc.gpsimd.snap(mreg, min_val=0, max_val=554)
        wo = nc.gpsimd.snap(wreg, min_val=0, max_val=160 - WW160)
        co = nc.gpsimd.snap(creg, min_val=0, max_val=W0 - WWIN)
        ro = nc.gpsimd.snap(rreg, min_val=0, max_val=ROWS - RWIN)
        nc.gpsimd.dma_start(
            protosw[:, :].rearrange("c (h w) -> c h w", w=WW160),
            d["protos"].ap().rearrange("c (h w) -> c h w", w=160)
            [:, :, bass.ds(wo, WW160)])
        nc.gpsimd.dma_start(awW[:, :],
                            d["awtp"].ap()[bass.ds(wo, WW160), bass.ds(mo, SWIN)])
        nc.gpsimd.dma_start(vww[:, :],
                            d["vwpad"].ap()[bass.ds(mo, SWIN), bass.ds(co, WWIN)])
        xt = d["xs"].ap().transpose([1, 0, 2])      # [270, 3, 3840]
        nc.gpsimd.dma_start(
            xw[:, :].rearrange("p (c w) -> p c w", c=3),
            xt[bass.ds(ro, RWIN), :, bass.ds(co, WWIN)])

    # coef matvec over the window: m160w[1, (h24, w28)]
    psM = ps.tile([1, MROWS * WW160], F32, tag="psM", name=f"psM{rep}", bufs=1)
    nc.tensor.matmul(psM[0:1, 0:512], coefT, protosw[:, 0:512],
                     start=True, stop=True)
    nc.tensor.matmul(psM[0:1, 512:MROWS * WW160], coefT,
                     protosw[:, 512:MROWS * WW160], start=True, stop=True)
    m160wf = tile1("m160wf", (1, MROWS * WW160))
    nc.scalar.copy(m160wf[:, :], psM[:, :])
    m160r = tile1("m160r", (MROWS, WW160))
    nc.sync.dma_start(
        m160r[:, :],
        m160wf[:, :].rearrange("q (h w) -> (q h) w", w=WW160))

    # step1: P1T[w28, j82] = sum_h m160r[h, w] * ahst[h, j]
    psP = ps.tile([WW160, SROWS], F32, tag="ps", name=f"psP{rep}")
    nc.tensor.matmul(psP[:, :], m160r[:, :], ahst[:, :], start=True, stop=True)
    p1 = tile1("p1", (WW160, SROWS))
    nc.scalar.copy(p1[:, :], psP[:, :])

    # step2: m640win[j82, i88] = sum_w P1T[w, j] * awW[w, i]
    psQ = ps.tile([SROWS, SWIN], F32, tag="ps", name=f"psQ{rep}")
    nc.tensor.matmul(psQ[:, :], p1[:, :], awW[:, :], start=True, stop=True)
    s_win = tile1("s_win", (SROWS, SWIN))
    act(s_win[:, :], psQ[:, :], AF.Sigmoid)

    # uT[i88, m270] = sum_p s_win[p, i] * r82[p, m]
    psU = ps.tile([SWIN, ROWS], F32, tag="ps", name=f"psU{rep}")
    nc.tensor.matmul(psU[:, :], s_win[:, :], r82[:, :], start=True, stop=True)
    uT = tile1("uT", (SWIN, ROWS))
    nc.scalar.copy(uT[:, :], psU[:, :])

    uTw = tile1("uTw", (SWIN, RWIN))
    with nc.gpsimd.register(f"ro2{rep}") as rreg:
        nc.gpsimd.reg_load(rreg, rw_i[0:1, 0:1])
        ro = nc.gpsimd.snap(rreg, min_val=0, max_val=ROWS - RWIN)
        nc.gpsimd.dma_start(uTw[:, :], uT[:, bass.ds(ro, RWIN)])

    if stage <= 3:
        ctx.close()
        return

    # ---------------- stage O: threshold + rect + multiply ----------------
    xcol = tile1("xcol", (128, WWIN))
    xiota = tile1("xiota", (128, WWIN))
    nc.sync.dma_start(xiota[:, :], d["xiota"].ap())
    ts(xcol[:, :], xiota[:, :], c0col[:, :], None, OP.add)
    cma = tile1("cma", (128, WWIN))
    ts(cma[:, :], xcol[:, :], fb0[:, :], 255.0, OP.is_ge, OP.mult)
    cmb = tile1("cmb", (128, WWIN))
    ts(cmb[:, :], xcol[:, :], fb2[:, :], None, OP.is_lt)
    cm255 = tile1("cm255", (128, WWIN))
    tt(cm255[:, :], cma[:, :], cmb[:, :], OP.mult)

    psW = ps.tile([RWIN, WWIN], F32, tag="psW", name=f"psW{rep}", bufs=1)
    nc.tensor.matmul(psW[:, 0:512], uTw[:, :], vww[:, 0:512],
                     start=True, stop=True)
    nc.tensor.matmul(psW[:, 512:WWIN], uTw[:, :], vww[:, 512:WWIN],
                     start=True, stop=True)
    sgn = tile1("sgn", (RWIN, WWIN))
    act(sgn[:, :], psW[:, :], AF.Sign, bias=-MASK_THR)

    rm = tile1("rm")
    rmb = tile1("rmb")
    ts(rm[:, :], riog[:, :], fb1[:, :], None, OP.is_ge)
    ts(rmb[:, :], riog[:, :], fb3[:, :], None, OP.is_lt)
    tt(rm[:, :], rm[:, :], rmb[:, :], OP.mult)
    bm = tile1("bm", (RWIN, WWIN))
    ts(bm[:, :], sgn[:, :], 0.0, rm[:, :], OP.max, OP.mult)
    bm3 = tile1("bm3", (RWIN, WWIN))
    tt(bm3[:, :], bm[:, :], cm255[:, :], OP.mult)

    res = tile1("res", (RWIN, 3 * WWIN))
    for ch in range(3):
        tt(res[:, WWIN * ch:WWIN * (ch + 1)],
           xw[:, WWIN * ch:WWIN * (ch + 1)], bm3[:, :], OP.mult)

    out_t = d["out"].ap().transpose([1, 0, 2])          # [270, 3, 3840]
    with nc.gpsimd.register(f"co2{rep}") as creg, \
            nc.gpsimd.register(f"ro3{rep}") as rreg:
        nc.gpsimd.reg_load(creg, c0_i[0:1, 0:1])
        nc.gpsimd.reg_load(rreg, rw_i[0:1, 0:1])
        co = nc.gpsimd.snap(creg, min_val=0, max_val=W0 - WWIN)
        ro = nc.gpsimd.snap(rreg, min_val=0, max_val=ROWS - RWIN)
        nc.gpsimd.dma_start(
            out_t[bass.ds(ro, RWIN), :, bass.ds(co, WWIN)],
            res[:, :].rearrange("p (c w) -> p c w", c=3))

    ctx.close()


# ---------------------------------------------------------------------------
# host orchestration
# ---------------------------------------------------------------------------

_NC_CACHE = None


def _get_nc():
    global _NC_CACHE
    if _NC_CACHE is None:
        _NC_CACHE = _build_nc()
    return _NC_CACHE


def _make_in_maps(x_raw, pred2, proto2, shared, percore):
    in_maps = []
    for c in range(N_CORES):
        pc = percore[c]
        ha = pc["ha"]
        in_maps.append({
            "pred": pred2,
            "xs": np.ascontiguousarray(x_raw[0, :, ROWS * c:ROWS * (c + 1), :]),
            "protos": np.ascontiguousarray(
                proto2[:, ha:ha + MROWS, :].reshape(32, MROWS * 160)),
            "ahst": pc["ahst"],
            "awtp": shared["awtp"],
            "r82": pc["r82"],
            "vwpad": shared["vwpad"],
            "ones1": shared["ones1"],
            "id1": shared["id1"],
            "xiota": shared["xiota"],
            "pio1": shared["pio1"],
            "pio66": shared["pio66"],
            "r0c": pc["r0c"],
        })
    return in_maps


def _numpy_fallback(x_raw, pred, proto):
    """Exact slow-path reference (only used if the rect exceeds the device
    windows, which cannot happen for in-distribution inputs)."""
    p = pred[0]
    boxes, cls, coef = p[:, :4], p[:, 4], p[:, 5:]
    s1 = np.maximum(1.0 / (1.0 + np.exp(-cls)) - 0.5, 0) + np.float32(0.001)
    mk = np.abs(coef).sum(-1)
    f = np.float32(640.0 if boxes.max() <= 1.2 else 1.0)
    dxdy = np.abs(boxes[:, :2] * f - 320.0) / 320.0
    cw = np.maximum(1.0 - 0.5 * (dxdy[:, 0] + dxdy[:, 1]), 0.0)
    a = int(np.argmax(s1 * mk * (0.5 + 0.5 * cw)))
    fcoef = coef[a]
    cx, cy, w, h = boxes[a]
    xyxy = np.clip(np.array([cx - w / 2, cy - h / 2, cx + w / 2, cy + h / 2],
                            np.float32), 0.0, IMGSZ - 1)
    fb = xyxy * np.array([W0 / IMGSZ, H0 / IMGSZ, W0 / IMGSZ, H0 / IMGSZ],
                         np.float32)
    Ah = _weight_mat(160, IMGSZ)
    Aw = _weight_mat(160, IMGSZ)
    Vh = _weight_mat(IMGSZ, H0)
    Vw = _weight_mat(IMGSZ, W0)
    m160 = (fcoef @ proto[0].reshape(32, -1)).reshape(160, 160)
    m640 = Ah.T @ m160 @ Aw
    s640 = 1.0 / (1.0 + np.exp(-m640))
    m_orig = (Vh.T @ s640 @ Vw).astype(np.float32)
    ys = np.arange(H0, dtype=np.float32)[:, None]
    xs = np.arange(W0, dtype=np.float32)[None, :]
    rect = (xs >= fb[0]) & (xs < fb[2]) & (ys >= fb[1]) & (ys < fb[3])
    bm = ((m_orig > MASK_THR) & rect).astype(np.float32)
    return (np.clip(x_raw * 255.0, 0.0, 255.0) * bm[None, None]).astype(np.float32)


def _covered(metas):
    """Check every rect pixel lies inside each core's written window."""
    fb0, fb1, fb2, fb3 = metas[0][1], metas[0][2], metas[0][3], metas[0][4]
    if fb2 <= fb0 or fb3 <= fb1:
        return True
    c0 = metas[0][5]
    cols = np.arange(W0, dtype=np.float32)
    csel = (cols >= fb0) & (cols < fb2)
    if csel.any():
        lo, hi = np.where(csel)[0][[0, -1]]
        if not (c0 <= lo and hi < c0 + WWIN):
            return False
    rows = np.arange(H0, dtype=np.float32)
    rsel = (rows >= fb1) & (rows < fb3)
    for c in range(N_CORES):
        sel = rsel[ROWS * c:ROWS * (c + 1)]
        if sel.any():
            rw = metas[c][6]
            lo, hi = np.where(sel)[0][[0, -1]]
            if not (rw <= lo and hi < rw + RWIN):
                return False
    return True


def kernel(x_raw, pred, proto):
    x_raw = np.ascontiguousarray(np.asarray(x_raw, dtype=np.float32))
    pred = np.ascontiguousarray(np.asarray(pred, dtype=np.float32))
    proto = np.ascontiguousarray(np.asarray(proto, dtype=np.float32))

    nc = _get_nc()
    shared, percore = _host_consts()
    pred2 = np.ascontiguousarray(pred[0])
    proto2 = proto[0]
    in_maps = _make_in_maps(x_raw, pred2, proto2, shared, percore)

    res = bass_utils.run_bass_kernel_spmd(nc, in_maps,
                                          core_ids=list(range(N_CORES)))

    metas = [res.results[c]["meta"][0] for c in range(N_CORES)]
    if not _covered(metas):
        return _numpy_fallback(x_raw, pred, proto)

    out = np.concatenate([res.results[c]["out"] for c in range(N_CORES)],
                         axis=1)          # [3, 2160, 3840]
    return out[None]


if __name__ == "__main__":
    import jax
    with jax.default_device(jax.devices("cpu")[0]):
        import reference as R
        inputs = R.setup_inputs()
        inputs = {k: np.asarray(v) for k, v in inputs.items()}
    out = kernel(**inputs)
    ref = np.load("/tmp/ref_out.npy")
    print("absmax:", np.abs(out - ref).max())
